# revision 2
# baseline (speedup 1.0000x reference)
"""CoAttention module kernel for Trainium2 (8 NeuronCores), v3.

Problem: B=4 pairs of (left, right) feature maps [B, C=2048, H=W=48].
Two attention directions per pair -> 8 independent attention problems,
one per core (data parallel, no cross-core communication).

Per core (qf = query features [C, HW], rf = reference features [C, HW]):
    Q = Wq @ qf + bq          [HC=256, HW=2304]
    K = Wk @ rf + bk          [HC=256, HW=2304]
    S = Q^T K                 [2304, 2304]
    P = softmax(S, axis=-1)
    O = V P^T, V = rf         [C, HW]

v3 changes over v2 (all microbenchmark-driven; PE is the bottleneck at
~0.227 ns/row bf16 + ~30-50 ns fixed cost per matmul):
  * AV computed in O^T layout [i, c]: stationary = P^T 128-wide i-chunks,
    moving = V^T 512-wide c-slices. 1296 uniform 512-row matmuls instead
    of 1440 (the 256-wide tail super wasted matmul fixed cost).
  * Softmax normalization ON DEVICE, folded into the AV PSUM->SBUF
    eviction: T^T[i,1] = acc^T ones via a 1-row fp32 matmul (acc as
    stationary!), alpha = 1/T via DVE reciprocal, ACT eviction applies
    scale=alpha per partition (i is the partition axis in O^T layout).
  * Output therefore ships as normalized fp16 [HW, C] (9.4 MB vs 18.9):
    host just transposes (free). No `sums` output, no host division.
  * fp16 for proj + S operands (softmax-sensitive), bf16 for V / P
    (P = exp(S - 64) can reach e^34: overflows fp16, fits bf16).
  * All matmuls rotate stationaries (same-stationary back-to-back
    measured 2.7x slower); weights/V^T pre-transposed on host (free).
"""

import sys

sys.path.insert(0, "/opt/trn_rl_repo")

import numpy as np
import ml_dtypes

import concourse.mybir as mybir
import concourse.tile as tile
from concourse import bacc
from concourse.bass_utils import run_bass_kernel_spmd

B, C, H, W = 4, 2048, 48, 48
HW = H * W  # 2304
HC = 256

F32 = mybir.dt.float32
F16 = mybir.dt.float16
BF16 = mybir.dt.bfloat16

NCC = C // 128  # 16 channel chunks
NHC = HC // 128  # 2 head-channel halves
NJT = HW // 128  # 18 j tiles
NCS = C // 512  # 4 c-slices for O^T moving dim
TAU = 64.0
# i-stripes == S^T supers (PSUM-bank sized free dim).
SUPERS = [(0, 512), (512, 512), (1024, 512), (1536, 512), (2048, 256)]
NS = len(SUPERS)

_CACHED_NC = None


def build_nc(reps=1):
    nc = bacc.Bacc("TRN2", target_bir_lowering=False, debug=False, num_devices=8)

    qf = nc.dram_tensor("qf", [C, HW], F16, kind="ExternalInput").ap()
    rf = nc.dram_tensor("rf", [C, HW], F16, kind="ExternalInput").ap()
    rfT = nc.dram_tensor("rfT", [HW, C], BF16, kind="ExternalInput").ap()
    WqT = nc.dram_tensor("WqT", [C, HC], F16, kind="ExternalInput").ap()
    WkT = nc.dram_tensor("WkT", [C, HC], F16, kind="ExternalInput").ap()
    bq = nc.dram_tensor("bq", [HC], F32, kind="ExternalInput").ap()
    bk = nc.dram_tensor("bk", [HC], F32, kind="ExternalInput").ap()
    # O^T [i, c], softmax-normalized on device.
    outT = nc.dram_tensor("outT", [HW, C], F16, kind="ExternalOutput").ap()

    with tile.TileContext(nc) as tc:
        for r in range(reps):
            build_tile_kernel(tc, outT, qf, rf, rfT, WqT, WkT, bq, bk)

    nc.compile()
    return nc


def build_tile_kernel(tc, outT, qf, rf, rfT, WqT, WkT, bq, bk):
    nc = tc.nc

    with (
        tc.tile_pool(name="persist", bufs=1) as persist,
        tc.tile_pool(name="consts", bufs=1) as consts,
        tc.tile_pool(name="wt", bufs=1) as wt_pool,
        tc.tile_pool(name="pt", bufs=2) as pt_pool,
        tc.tile_pool(name="streamx", bufs=6) as streamx,
        tc.tile_pool(name="sums", bufs=2) as sums_pool,
        tc.tile_pool(name="alpha", bufs=2) as alpha_pool,
        tc.tile_pool(name="sbuf_o", bufs=3) as pool_o,
        tc.tile_pool(name="projpsum", bufs=1, space="PSUM") as proj_psum,
        tc.tile_pool(name="spsum", bufs=2, space="PSUM") as s_psum,
        tc.tile_pool(name="opsum", bufs=3, space="PSUM") as o_psum,
        tc.tile_pool(name="tpsum", bufs=1, space="PSUM") as t_psum,
    ):
        # Persistent tensors.
        VT = persist.tile([128, NJT, C], BF16, tag="VT")  # VT[jp, jc, c]
        Q_sb = persist.tile([128, NHC, HW], F16, tag="Q")  # [hp, h, i]
        K_sb = persist.tile([128, NHC, HW], F16, tag="K")  # [hp, h, j]
        WqT_sb = wt_pool.tile([128, NCC, HC], F16, tag="WqT")
        WkT_sb = wt_pool.tile([128, NCC, HC], F16, tag="WkT")

        bq_t = consts.tile([128, NHC], F32, tag="bq")
        bk_t = consts.tile([128, NHC], F32, tag="bk")
        negtau = consts.tile([128, 1], F32, tag="negtau")
        ones = consts.tile([128, 1], F32, tag="ones")
        nc.vector.memset(negtau[:], -TAU)
        nc.vector.memset(ones[:], 1.0)
        nc.sync.dma_start(out=WqT_sb[:], in_=WqT.rearrange("(cc p) h -> p cc h", p=128))
        nc.sync.dma_start(out=WkT_sb[:], in_=WkT.rearrange("(cc p) h -> p cc h", p=128))
        nc.sync.dma_start(out=bq_t[:], in_=bq.rearrange("(h p) -> p h", p=128))
        nc.sync.dma_start(out=bk_t[:], in_=bk.rearrange("(h p) -> p h", p=128))

        PTs = {}
        accs = {}
        alphas = {}

        def proj_stripe(is_k, s):
            """Project one j/i stripe of K (from rf) or Q (from qf)."""
            src = rf if is_k else qf
            WT = WkT_sb if is_k else WqT_sb
            dst = K_sb if is_k else Q_sb
            bias = bk_t if is_k else bq_t
            j0, jw = SUPERS[s]
            tag = "k" if is_k else "q"
            pp = [
                proj_psum.tile([128, 512], F32, tag=f"pp{h}", name=f"pp_{tag}_{s}_{h}")
                for h in range(NHC)
            ]
            for cc in range(NCC):
                xt = streamx.tile([128, 512], F16, tag="xt", name=f"xt{tag}{s}{cc}")
                nc.sync.dma_start(
                    out=xt[:, :jw], in_=src[cc * 128 : (cc + 1) * 128, j0 : j0 + jw]
                )
                for h in range(NHC):
                    nc.tensor.matmul(
                        pp[h][:, :jw],
                        WT[:, cc, h * 128 : (h + 1) * 128],
                        xt[:, :jw],
                        start=(cc == 0),
                        stop=(cc == NCC - 1),
                    )
            for h in range(NHC):
                nc.scalar.activation(
                    dst[:, h, j0 : j0 + jw],
                    pp[h][:, :jw],
                    mybir.ActivationFunctionType.Identity,
                    bias=bias[:, h : h + 1],
                    scale=1.0,
                )

        def st_begin(s):
            i0, iw = SUPERS[s]
            PTs[s] = pt_pool.tile([128, NJT, 512], BF16, tag="PT", name=f"PT_{s}")
            accs[s] = sums_pool.tile([128, 512], F32, tag="acc", name=f"acc_{s}")

        def st_unit(s, jt):
            """S^T tile (j-tile jt) for i-stripe s: matmul + exp + sum-accum."""
            i0, iw = SUPERS[s]
            acc = accs[s]
            ps = s_psum.tile([128, 512], F32, tag="ps", name=f"ps_{s}_{jt}")
            for h in range(NHC):
                nc.tensor.matmul(
                    ps[:, :iw],
                    K_sb[:, h, jt * 128 : (jt + 1) * 128],
                    Q_sb[:, h, i0 : i0 + iw],
                    start=(h == 0),
                    stop=(h == NHC - 1),
                )
            nc.scalar.activation(
                PTs[s][:, jt, :iw],
                ps[:, :iw],
                mybir.ActivationFunctionType.Exp,
                bias=negtau[:],
                scale=1.0,
            )
            if jt == 0:
                nc.vector.tensor_copy(acc[:, :iw], PTs[s][:, 0, :iw])
            else:
                nc.vector.tensor_tensor(
                    acc[:, :iw],
                    acc[:, :iw],
                    PTs[s][:, jt, :iw],
                    op=mybir.AluOpType.add,
                )

        def st_finish(s):
            """alpha^T[i,1] = 1 / (ones^T acc)^T per 128-wide i-chunk: the
            softmax denominator, transposed onto partitions by using acc as
            the matmul stationary."""
            i0, iw = SUPERS[s]
            nic = iw // 128
            al = alpha_pool.tile([128, 4], F32, tag="al", name=f"al_{s}")
            alphas[s] = al
            for ic in range(nic):
                tp = t_psum.tile([128, 1], F32, tag="tp", name=f"tp_{s}_{ic}")
                nc.tensor.matmul(
                    tp[:],
                    accs[s][:, ic * 128 : (ic + 1) * 128],
                    ones[:],
                    start=True,
                    stop=True,
                )
                nc.vector.reciprocal(al[:, ic : ic + 1], tp[:])

        def av_unit(s, u):
            """O^T chunk: i-chunk ic of super s, c-slice cs (512 wide).
            18-deep accumulation over j; eviction applies 1/T scale."""
            i0, iw = SUPERS[s]
            nic = iw // 128
            ic, cs = u // NCS, u % NCS
            po = o_psum.tile([128, 512], F32, tag="po", name=f"po_{s}_{u}")
            for jc in range(NJT):
                nc.tensor.matmul(
                    po[:],
                    PTs[s][:, jc, ic * 128 : (ic + 1) * 128],
                    VT[:, jc, cs * 512 : (cs + 1) * 512],
                    start=(jc == 0),
                    stop=(jc == NJT - 1),
                )
            osb = osbs[ic]
            nc.scalar.activation(
                osb[:, cs * 512 : (cs + 1) * 512],
                po[:],
                mybir.ActivationFunctionType.Identity,
                bias=0.0,
                scale=alphas[s][:, ic : ic + 1],
            )
            if cs == NCS - 1:
                nc.sync.dma_start(
                    out=outT[i0 + ic * 128 : i0 + (ic + 1) * 128, :], in_=osb[:]
                )

        def vt_dma(jc):
            nc.sync.dma_start(
                out=VT[:, jc, :], in_=rfT[jc * 128 : (jc + 1) * 128, :]
            )

        # ---- Prologue: K projection (rf streams first), then Q stripes 0/1
        # chase their qf DMAs; VT DMA rides between. S^T stripe 0 closes it.
        for s in range(NS):
            proj_stripe(True, s)
        proj_stripe(False, 0)
        for jc in range(NJT // 2):
            vt_dma(jc)
        proj_stripe(False, 1)
        for jc in range(NJT // 2, NJT):
            vt_dma(jc)
        st_begin(0)
        for jt in range(NJT):
            st_unit(0, jt)
        st_finish(0)

        # ---- Steady state: AV super s (nic*NCS units), with S^T (s+1) and
        # Q-proj (s+2) interleaved between its units.
        for s in range(NS):
            i0, iw = SUPERS[s]
            nic = iw // 128
            nunits = nic * NCS
            osbs = [
                pool_o.tile([128, C], F16, tag="osb", name=f"o_{s}_{ic}")
                for ic in range(nic)
            ]
            sched = {}
            if s + 1 < NS:
                st_begin(s + 1)
                for jt in range(NJT):
                    sched.setdefault((jt * nunits) // (NJT + 2), []).append(
                        ("ST", s + 1, jt)
                    )
                sched.setdefault(nunits - 3, []).append(("STF", s + 1, 0))
            if s + 2 < NS:
                sched.setdefault(nunits - 2, []).append(("QP", s + 2, 0))
            for u in range(nunits):
                av_unit(s, u)
                for kind, a1, a2 in sched.get(u, []):
                    if kind == "ST":
                        st_unit(a1, a2)
                    elif kind == "STF":
                        st_finish(a1)
                    else:
                        proj_stripe(False, a1)


def get_nc():
    global _CACHED_NC
    if _CACHED_NC is None:
        _CACHED_NC = build_nc()
    return _CACHED_NC


def make_in_maps(inputs):
    """Host-side prep: shard 8 (batch, direction) problems, pre-transpose
    weights/V and cast to the PE dtypes."""
    left = np.ascontiguousarray(
        np.asarray(inputs["left_features"], dtype=np.float32)
    ).reshape(B, C, HW)
    right = np.ascontiguousarray(
        np.asarray(inputs["right_features"], dtype=np.float32)
    ).reshape(B, C, HW)
    Wq = np.asarray(inputs["Wq"], dtype=np.float32)
    Wk = np.asarray(inputs["Wk"], dtype=np.float32)
    bq = np.ascontiguousarray(np.asarray(inputs["bq"], dtype=np.float32))
    bk = np.ascontiguousarray(np.asarray(inputs["bk"], dtype=np.float32))

    WqT16 = np.ascontiguousarray(Wq.T).astype(np.float16)
    WkT16 = np.ascontiguousarray(Wk.T).astype(np.float16)
    l16 = [np.ascontiguousarray(left[b]).astype(np.float16) for b in range(B)]
    r16 = [np.ascontiguousarray(right[b]).astype(np.float16) for b in range(B)]
    lT = [
        np.ascontiguousarray(left[b].T).astype(ml_dtypes.bfloat16) for b in range(B)
    ]
    rT = [
        np.ascontiguousarray(right[b].T).astype(ml_dtypes.bfloat16) for b in range(B)
    ]

    maps = []
    # cores 0..3: weighted_r for batch b (query=left, ref=right)
    for b in range(B):
        maps.append({"qf": l16[b], "rf": r16[b], "rfT": rT[b],
                     "WqT": WqT16, "WkT": WkT16, "bq": bq, "bk": bk})
    # cores 4..7: weighted_l for batch b (query=right, ref=left)
    for b in range(B):
        maps.append({"qf": r16[b], "rf": l16[b], "rfT": lT[b],
                     "WqT": WqT16, "WkT": WkT16, "bq": bq, "bk": bk})
    return maps


def kernel(left_features, right_features, Wq, bq, Wk, bk):
    inputs = {"left_features": left_features, "right_features": right_features,
              "Wq": Wq, "bq": bq, "Wk": Wk, "bk": bk}
    in_maps = make_in_maps(inputs)
    nc = get_nc()
    res = run_bass_kernel_spmd(nc, in_maps, core_ids=list(range(8)))

    def norm_out(i):
        oT = np.asarray(res.results[i]["outT"], dtype=np.float32)  # [HW, C]
        return np.ascontiguousarray(oT.T)  # [C, HW], already normalized

    weighted_r = np.stack([norm_out(b) for b in range(B)]).reshape(B, C, H, W)
    weighted_l = np.stack([norm_out(B + b) for b in range(B)]).reshape(B, C, H, W)
    left4 = np.asarray(left_features, dtype=np.float32).reshape(B, C, H, W)
    right4 = np.asarray(right_features, dtype=np.float32).reshape(B, C, H, W)
    left_attended = np.concatenate([left4, weighted_l], axis=1)
    right_attended = np.concatenate([right4, weighted_r], axis=1)
    return (left_attended, right_attended)


# revision 9
# speedup vs baseline: 35.7238x; 35.7238x over previous
"""CoAttention module kernel for Trainium2 (8 NeuronCores), v3.

Problem: B=4 pairs of (left, right) feature maps [B, C=2048, H=W=48].
Two attention directions per pair -> 8 independent attention problems,
one per core (data parallel, no cross-core communication).

Per core (qf = query features [C, HW], rf = reference features [C, HW]):
    Q = Wq @ qf + bq          [HC=256, HW=2304]
    K = Wk @ rf + bk          [HC=256, HW=2304]
    S = Q^T K                 [2304, 2304]
    P = softmax(S, axis=-1)
    O = V P^T, V = rf         [C, HW]

v3 changes over v2 (all microbenchmark-driven; PE is the bottleneck at
~0.227 ns/row bf16 + ~30-50 ns fixed cost per matmul):
  * AV computed in O^T layout [i, c]: stationary = P^T 128-wide i-chunks,
    moving = V^T 512-wide c-slices. 1296 uniform 512-row matmuls instead
    of 1440 (the 256-wide tail super wasted matmul fixed cost).
  * Softmax normalization ON DEVICE, folded into the AV PSUM->SBUF
    eviction: T^T[i,1] = acc^T ones via a 1-row fp32 matmul (acc as
    stationary!), alpha = 1/T via DVE reciprocal, ACT eviction applies
    scale=alpha per partition (i is the partition axis in O^T layout).
  * Output therefore ships as normalized fp16 [HW, C] (9.4 MB vs 18.9):
    host just transposes (free). No `sums` output, no host division.
  * fp16 for proj + S operands (softmax-sensitive), bf16 for V / P
    (P = exp(S - 64) can reach e^34: overflows fp16, fits bf16).
  * All matmuls rotate stationaries (same-stationary back-to-back
    measured 2.7x slower); weights/V^T pre-transposed on host (free).
"""

import os
import sys

sys.path.insert(0, "/opt/trn_rl_repo")

import numpy as np
import ml_dtypes

# Debug switch: skip on-device normalization (timing bisection only —
# output is then unnormalized bf16 and correctness checks fail).
_NO_NORM = os.environ.get("KERNEL_NO_NORM", "0") == "1"

import concourse.mybir as mybir
import concourse.tile as tile
from concourse import bacc
from concourse.bass_utils import run_bass_kernel_spmd

B, C, H, W = 4, 2048, 48, 48
HW = H * W  # 2304
HC = 256

F32 = mybir.dt.float32
F16 = mybir.dt.float16
BF16 = mybir.dt.bfloat16

NCC = C // 128  # 16 channel chunks
NHC = HC // 128  # 2 head-channel halves
NJT = HW // 128  # 18 j tiles
NCS = C // 512  # 4 c-slices for O^T moving dim
TAU = 64.0
# i-stripes == S^T supers (PSUM-bank sized free dim).
SUPERS = [(0, 512), (512, 512), (1024, 512), (1536, 512), (2048, 256)]
NS = len(SUPERS)

_CACHED_NC = None


def build_nc(reps=1):
    nc = bacc.Bacc("TRN2", target_bir_lowering=False, debug=False, num_devices=8)

    qf = nc.dram_tensor("qf", [C, HW], F16, kind="ExternalInput").ap()
    rf = nc.dram_tensor("rf", [C, HW], F16, kind="ExternalInput").ap()
    rfT = nc.dram_tensor("rfT", [HW, C], BF16, kind="ExternalInput").ap()
    WqT = nc.dram_tensor("WqT", [C, HC], F16, kind="ExternalInput").ap()
    WkT = nc.dram_tensor("WkT", [C, HC], F16, kind="ExternalInput").ap()
    bq = nc.dram_tensor("bq", [HC], F32, kind="ExternalInput").ap()
    bk = nc.dram_tensor("bk", [HC], F32, kind="ExternalInput").ap()
    # O^T [i, c], softmax-normalized on device.
    outT = nc.dram_tensor(
        "outT", [HW, C], BF16 if _NO_NORM else F16, kind="ExternalOutput"
    ).ap()

    with tile.TileContext(nc) as tc:
        for r in range(reps):
            build_tile_kernel(tc, outT, qf, rf, rfT, WqT, WkT, bq, bk)

    nc.compile()
    return nc


def build_tile_kernel(tc, outT, qf, rf, rfT, WqT, WkT, bq, bk):
    nc = tc.nc

    with (
        tc.tile_pool(name="persist", bufs=1) as persist,
        tc.tile_pool(name="consts", bufs=1) as consts,
        tc.tile_pool(name="wt", bufs=1) as wt_pool,
        tc.tile_pool(name="pt", bufs=2) as pt_pool,
        tc.tile_pool(name="streamx", bufs=6) as streamx,
        tc.tile_pool(name="sums", bufs=2) as sums_pool,
        tc.tile_pool(name="alpha", bufs=2) as alpha_pool,
        tc.tile_pool(name="sbuf_o", bufs=3) as pool_o,
        tc.tile_pool(name="projpsum", bufs=1, space="PSUM") as proj_psum,
        tc.tile_pool(name="spsum", bufs=2, space="PSUM") as s_psum,
        tc.tile_pool(name="opsum", bufs=3, space="PSUM") as o_psum,
        tc.tile_pool(name="tpsum", bufs=1, space="PSUM") as t_psum,
    ):
        # Persistent tensors.
        VT = persist.tile([128, NJT, C], BF16, tag="VT")  # VT[jp, jc, c]
        Q_sb = persist.tile([128, NHC, HW], F16, tag="Q")  # [hp, h, i]
        K_sb = persist.tile([128, NHC, HW], F16, tag="K")  # [hp, h, j]
        WqT_sb = wt_pool.tile([128, NCC, HC], F16, tag="WqT")
        WkT_sb = wt_pool.tile([128, NCC, HC], F16, tag="WkT")

        bq_t = consts.tile([128, NHC], F32, tag="bq")
        bk_t = consts.tile([128, NHC], F32, tag="bk")
        negtau = consts.tile([128, 1], F32, tag="negtau")
        ones = consts.tile([128, 1], BF16, tag="ones")
        nc.vector.memset(negtau[:], -TAU)
        nc.vector.memset(ones[:], 1.0)
        nc.sync.dma_start(out=WqT_sb[:], in_=WqT.rearrange("(cc p) h -> p cc h", p=128))
        nc.sync.dma_start(out=WkT_sb[:], in_=WkT.rearrange("(cc p) h -> p cc h", p=128))
        nc.sync.dma_start(out=bq_t[:], in_=bq.rearrange("(h p) -> p h", p=128))
        nc.sync.dma_start(out=bk_t[:], in_=bk.rearrange("(h p) -> p h", p=128))

        PTs = {}
        accs = {}
        alphas = {}

        def proj_stripe(is_k, s):
            """Project one j/i stripe of K (from rf) or Q (from qf)."""
            src = rf if is_k else qf
            WT = WkT_sb if is_k else WqT_sb
            dst = K_sb if is_k else Q_sb
            bias = bk_t if is_k else bq_t
            j0, jw = SUPERS[s]
            tag = "k" if is_k else "q"
            pp = [
                proj_psum.tile([128, 512], F32, tag=f"pp{h}", name=f"pp_{tag}_{s}_{h}")
                for h in range(NHC)
            ]
            for cc in range(NCC):
                xt = streamx.tile([128, 512], F16, tag="xt", name=f"xt{tag}{s}{cc}")
                nc.sync.dma_start(
                    out=xt[:, :jw], in_=src[cc * 128 : (cc + 1) * 128, j0 : j0 + jw]
                )
                for h in range(NHC):
                    nc.tensor.matmul(
                        pp[h][:, :jw],
                        WT[:, cc, h * 128 : (h + 1) * 128],
                        xt[:, :jw],
                        start=(cc == 0),
                        stop=(cc == NCC - 1),
                    )
            for h in range(NHC):
                nc.scalar.activation(
                    dst[:, h, j0 : j0 + jw],
                    pp[h][:, :jw],
                    mybir.ActivationFunctionType.Identity,
                    bias=bias[:, h : h + 1],
                    scale=1.0,
                )

        def st_begin(s):
            i0, iw = SUPERS[s]
            PTs[s] = pt_pool.tile([128, NJT, 512], BF16, tag="PT", name=f"PT_{s}")
            accs[s] = sums_pool.tile([128, 512], F32, tag="acc", name=f"acc_{s}")

        def st_unit(s, jt):
            """S^T tile (j-tile jt) for i-stripe s: matmul + exp + sum-accum."""
            i0, iw = SUPERS[s]
            acc = accs[s]
            ps = s_psum.tile([128, 512], F32, tag="ps", name=f"ps_{s}_{jt}")
            for h in range(NHC):
                nc.tensor.matmul(
                    ps[:, :iw],
                    K_sb[:, h, jt * 128 : (jt + 1) * 128],
                    Q_sb[:, h, i0 : i0 + iw],
                    start=(h == 0),
                    stop=(h == NHC - 1),
                )
            nc.scalar.activation(
                PTs[s][:, jt, :iw],
                ps[:, :iw],
                mybir.ActivationFunctionType.Exp,
                bias=negtau[:],
                scale=1.0,
            )
            if jt == 0:
                nc.vector.tensor_copy(acc[:, :iw], PTs[s][:, 0, :iw])
            else:
                nc.vector.tensor_tensor(
                    acc[:, :iw],
                    acc[:, :iw],
                    PTs[s][:, jt, :iw],
                    op=mybir.AluOpType.add,
                )

        def st_finish(s):
            """alpha^T[i,1] = 1 / (ones^T acc)^T per 128-wide i-chunk: the
            softmax denominator, transposed onto partitions by using acc as
            the matmul stationary. acc is cast to bf16 first — f32 matmul
            weight loads measured pathologically slow (~16 us each!), and
            bf16 partials only perturb T by ~0.03% rms after the 128-sum."""
            i0, iw = SUPERS[s]
            nic = iw // 128
            al = alpha_pool.tile([128, 4], F32, tag="al", name=f"al_{s}")
            alphas[s] = al
            if _NO_NORM:
                return
            a16 = sums_pool.tile([128, 512], BF16, tag="acc16", name=f"a16_{s}")
            nc.vector.tensor_copy(a16[:, :iw], accs[s][:, :iw])
            for ic in range(nic):
                tp = t_psum.tile([128, 1], F32, tag="tp", name=f"tp_{s}_{ic}")
                nc.tensor.matmul(
                    tp[:],
                    a16[:, ic * 128 : (ic + 1) * 128],
                    ones[:],
                    start=True,
                    stop=True,
                )
                nc.vector.reciprocal(al[:, ic : ic + 1], tp[:])

        def av_unit(s, u):
            """O^T chunk: i-chunk ic of super s, c-slice cs (512 wide).
            18-deep accumulation over j; eviction applies 1/T scale."""
            i0, iw = SUPERS[s]
            nic = iw // 128
            ic, cs = u // NCS, u % NCS
            po = o_psum.tile([128, 512], F32, tag="po", name=f"po_{s}_{u}")
            for jc in range(NJT):
                nc.tensor.matmul(
                    po[:],
                    PTs[s][:, jc, ic * 128 : (ic + 1) * 128],
                    VT[:, jc, cs * 512 : (cs + 1) * 512],
                    start=(jc == 0),
                    stop=(jc == NJT - 1),
                )
            osb = osbs[ic]
            nc.scalar.activation(
                osb[:, cs * 512 : (cs + 1) * 512],
                po[:],
                mybir.ActivationFunctionType.Identity,
                bias=0.0,
                scale=1.0 if _NO_NORM else alphas[s][:, ic : ic + 1],
            )
            if cs == NCS - 1:
                nc.sync.dma_start(
                    out=outT[i0 + ic * 128 : i0 + (ic + 1) * 128, :], in_=osb[:]
                )

        def vt_dma(jc):
            nc.sync.dma_start(
                out=VT[:, jc, :], in_=rfT[jc * 128 : (jc + 1) * 128, :]
            )

        # ---- Prologue: K projection (rf streams first), then Q stripes 0/1
        # chase their qf DMAs; VT DMA rides between. S^T stripe 0 closes it.
        for s in range(NS):
            proj_stripe(True, s)
        proj_stripe(False, 0)
        for jc in range(NJT // 2):
            vt_dma(jc)
        proj_stripe(False, 1)
        for jc in range(NJT // 2, NJT):
            vt_dma(jc)
        st_begin(0)
        for jt in range(NJT):
            st_unit(0, jt)
        st_finish(0)

        # ---- Steady state: AV super s (nic*NCS units), with S^T (s+1) and
        # Q-proj (s+2) interleaved between its units.
        for s in range(NS):
            i0, iw = SUPERS[s]
            nic = iw // 128
            nunits = nic * NCS
            osbs = [
                pool_o.tile([128, C], BF16 if _NO_NORM else F16, tag="osb",
                            name=f"o_{s}_{ic}")
                for ic in range(nic)
            ]
            sched = {}
            if s + 1 < NS:
                st_begin(s + 1)
                for jt in range(NJT):
                    sched.setdefault((jt * nunits) // (NJT + 2), []).append(
                        ("ST", s + 1, jt)
                    )
                sched.setdefault(nunits - 3, []).append(("STF", s + 1, 0))
            if s + 2 < NS:
                sched.setdefault(nunits - 2, []).append(("QP", s + 2, 0))
            for u in range(nunits):
                av_unit(s, u)
                for kind, a1, a2 in sched.get(u, []):
                    if kind == "ST":
                        st_unit(a1, a2)
                    elif kind == "STF":
                        st_finish(a1)
                    else:
                        proj_stripe(False, a1)


def get_nc():
    global _CACHED_NC
    if _CACHED_NC is None:
        _CACHED_NC = build_nc()
    return _CACHED_NC


def make_in_maps(inputs):
    """Host-side prep: shard 8 (batch, direction) problems, pre-transpose
    weights/V and cast to the PE dtypes."""
    left = np.ascontiguousarray(
        np.asarray(inputs["left_features"], dtype=np.float32)
    ).reshape(B, C, HW)
    right = np.ascontiguousarray(
        np.asarray(inputs["right_features"], dtype=np.float32)
    ).reshape(B, C, HW)
    Wq = np.asarray(inputs["Wq"], dtype=np.float32)
    Wk = np.asarray(inputs["Wk"], dtype=np.float32)
    bq = np.ascontiguousarray(np.asarray(inputs["bq"], dtype=np.float32))
    bk = np.ascontiguousarray(np.asarray(inputs["bk"], dtype=np.float32))

    WqT16 = np.ascontiguousarray(Wq.T).astype(np.float16)
    WkT16 = np.ascontiguousarray(Wk.T).astype(np.float16)
    l16 = [np.ascontiguousarray(left[b]).astype(np.float16) for b in range(B)]
    r16 = [np.ascontiguousarray(right[b]).astype(np.float16) for b in range(B)]
    lT = [
        np.ascontiguousarray(left[b].T).astype(ml_dtypes.bfloat16) for b in range(B)
    ]
    rT = [
        np.ascontiguousarray(right[b].T).astype(ml_dtypes.bfloat16) for b in range(B)
    ]

    maps = []
    # cores 0..3: weighted_r for batch b (query=left, ref=right)
    for b in range(B):
        maps.append({"qf": l16[b], "rf": r16[b], "rfT": rT[b],
                     "WqT": WqT16, "WkT": WkT16, "bq": bq, "bk": bk})
    # cores 4..7: weighted_l for batch b (query=right, ref=left)
    for b in range(B):
        maps.append({"qf": r16[b], "rf": l16[b], "rfT": lT[b],
                     "WqT": WqT16, "WkT": WkT16, "bq": bq, "bk": bk})
    return maps


def kernel(left_features, right_features, Wq, bq, Wk, bk):
    inputs = {"left_features": left_features, "right_features": right_features,
              "Wq": Wq, "bq": bq, "Wk": Wk, "bk": bk}
    in_maps = make_in_maps(inputs)
    nc = get_nc()
    res = run_bass_kernel_spmd(nc, in_maps, core_ids=list(range(8)))

    def norm_out(i):
        oT = np.asarray(res.results[i]["outT"], dtype=np.float32)  # [HW, C]
        return np.ascontiguousarray(oT.T)  # [C, HW], already normalized

    weighted_r = np.stack([norm_out(b) for b in range(B)]).reshape(B, C, H, W)
    weighted_l = np.stack([norm_out(B + b) for b in range(B)]).reshape(B, C, H, W)
    left4 = np.asarray(left_features, dtype=np.float32).reshape(B, C, H, W)
    right4 = np.asarray(right_features, dtype=np.float32).reshape(B, C, H, W)
    left_attended = np.concatenate([left4, weighted_l], axis=1)
    right_attended = np.concatenate([right4, weighted_r], axis=1)
    return (left_attended, right_attended)
